# revision 19
# baseline (speedup 1.0000x reference)
"""GRU encoder (Keras reset_after=True) on 8 Trainium2 NeuronCores.

Strategy: data-parallel over batch (8 rows/core). Per core:
  - bulk indirect-DMA gather of W_in rows (z/r biases pre-folded on host) into
    SBUF, cast fp16, rearranged SBUF->SBUF into a per-chunk x-projection
    buffer in the scan's "sparse group" layout (batch b of column-group g on
    partition 32g+b)
  - 256-step sequential scan:
      rec = h @ W_rec as out[M=batch, N=cols] with PE column-group tiling
      (tile_position -> 4 concurrent matmul streams), fp16 operands
      gates (sigmoid/tanh/blend) on DVE/ACT in the same sparse layout
      h^T for the next step's stationary operand rebuilt with 8 small
      identity matmuls on the PE
Host does all layout prep (weight tiling/casting, index tiles, initial h/h^T).
"""

import numpy as np

import concourse.mybir as mybir
from concourse.bacc import Bacc
from concourse.bass import IndirectOffsetOnAxis
from concourse.bass_utils import run_bass_kernel_spmd
from concourse.tile import TileContext

B, T, U, V = 64, 256, 1024, 10000
NCORES = 8
BL = B // NCORES          # 8 batch rows per core
G3 = 3 * U                # 3072
KT = U // 128             # 8 contraction k-tiles
NG = 4                    # PE column groups
GW = G3 // NG             # 768 rec columns per group (z|r|h x 256)
UC = U // NG              # 256 hidden units per group
CH = 16                   # scan steps per gather chunk
NCH = T // CH

FP32 = mybir.dt.float32
FP16 = mybir.dt.float16
I32 = mybir.dt.int32
AF = mybir.ActivationFunctionType


def build_bass(n_steps=T):
    nch = n_steps // CH
    nc = Bacc()
    win = nc.dram_tensor("win", [V, G3], FP32, kind="ExternalInput")
    wk = nc.dram_tensor("wk", [128, KT * G3], FP16, kind="ExternalInput")
    idx = nc.dram_tensor("idx", [128, nch], I32, kind="ExternalInput")
    h0s = nc.dram_tensor("h0s", [128, UC], FP32, kind="ExternalInput")
    h0t = nc.dram_tensor("h0t", [128, KT * BL], FP16, kind="ExternalInput")
    idn = nc.dram_tensor("idn", [128, BL], FP16, kind="ExternalInput")
    brh = nc.dram_tensor("brh", [1, U], FP16, kind="ExternalInput")
    out = nc.dram_tensor("out", [BL, n_steps, U], FP32, kind="ExternalOutput")

    with TileContext(nc) as tc:
        with (
            tc.tile_pool(name="const", bufs=1) as cpool,
            tc.tile_pool(name="gather", bufs=2) as gpool,
            tc.tile_pool(name="work", bufs=2) as wpool,
            tc.tile_pool(name="state", bufs=2) as spool,
            tc.tile_pool(name="psum", bufs=1, space="PSUM") as ppool,
        ):
            wk_sb = cpool.tile([128, KT * G3], FP16, name="wk_sb")
            nc.sync.dma_start(wk_sb[:], wk[:])
            idx_sb = cpool.tile([128, nch], I32, name="idx_sb")
            nc.sync.dma_start(idx_sb[:], idx[:])
            idn_sb = cpool.tile([128, BL], FP16, name="idn_sb")
            nc.sync.dma_start(idn_sb[:], idn[:])
            brh_sb = cpool.tile([1, U], FP16, name="brh_sb")
            nc.sync.dma_start(brh_sb[:], brh[:])
            ones_sb = cpool.tile([1, BL], FP16, name="ones_sb")
            nc.vector.memset(ones_sb[:], 1.0)
            h_first = cpool.tile([128, UC], FP32, name="h_first")
            nc.sync.dma_start(h_first[:], h0s[:])
            hT_first = cpool.tile([128, KT * BL], FP16, name="hT_first")
            nc.sync.dma_start(hT_first[:], h0t[:])

            # Persistent double-buffered tiles. These are only partially
            # written (8 of every 32 partitions), so they must be long-lived
            # tensors (memset once) rather than pool-cycled tiles, or reads of
            # the unwritten lanes alias dead slot contents.
            xc_bufs = []
            for i in range(2):
                xcb = cpool.tile([128, CH * GW], FP16, name=f"xcbuf{i}", tag=f"xcbuf{i}")
                nc.vector.memset(xcb[:], 0.0)
                xc_bufs.append(xcb)
            rec_bufs = []
            pT_bufs = []
            for i in range(2):
                recb = ppool.tile([128, GW], FP32, name=f"recbuf{i}", tag=f"recbuf{i}")
                nc.vector.memset(recb[:], 0.0)
                rec_bufs.append(recb)
                pTb = ppool.tile([128, KT * BL], FP32, name=f"pTbuf{i}", tag=f"pTbuf{i}")
                pT_bufs.append(pTb)

            # gather (W_in + bias)[token] -> fp16 -> sparse-group layout.
            # Emitted interleaved with the scan: chunk c+1 is emitted right
            # after the last scan read of its target buffer (program order
            # carries the write-after-read constraint for Tile).
            def emit_chunk(c):
                gt = gpool.tile([128, G3], FP32, name="gt", tag="gt")
                nc.gpsimd.indirect_dma_start(
                    out=gt[:],
                    out_offset=None,
                    in_=win[:, :],
                    in_offset=IndirectOffsetOnAxis(ap=idx_sb[:, c : c + 1], axis=0),
                )
                gt16 = gpool.tile([128, G3], FP16, name="gt16", tag="gt16")
                nc.scalar.copy(out=gt16[:], in_=gt[:])
                xc = xc_bufs[c % 2]
                for dt in range(CH):
                    s4 = gt16[8 * dt : 8 * dt + 8, :].rearrange(
                        "b (gate g j) -> b gate g j", gate=3, g=NG
                    )
                    for g in range(NG):
                        nc.sync.dma_start(
                            out=xc[32 * g : 32 * g + BL, dt * GW : (dt + 1) * GW].rearrange(
                                "b (gate j) -> b gate j", gate=3
                            ),
                            in_=s4[:, :, g, :],
                        )

            # ---- sequential scan ----
            emit_chunk(0)
            if nch > 1:
                emit_chunk(1)
            h_prev, hT_prev = h_first, hT_first
            for t in range(n_steps):
                if t >= CH and t % CH == 0 and t // CH + 1 < nch:
                    emit_chunk(t // CH + 1)
                xc = xc_bufs[(t // CH) % 2]
                xo = (t % CH) * GW
                rec = rec_bufs[t % 2]
                # z|r columns first so sigmoid can overlap the h-column matmuls
                for k in range(KT):
                    for g in range(NG):
                        o = k * G3 + g * GW
                        nc.tensor.matmul(
                            rec[32 * g : 32 * g + BL, 0:512],
                            lhsT=hT_prev[:, BL * k : BL * k + BL],
                            rhs=wk_sb[:, o : o + 512],
                            start=(k == 0),
                            stop=(k == KT - 1),
                            skip_group_check=True,
                            tile_position=(0, 32 * g),
                        )
                for k in range(KT):
                    for g in range(NG):
                        o = k * G3 + g * GW + 512
                        nc.tensor.matmul(
                            rec[32 * g : 32 * g + BL, 512:768],
                            lhsT=hT_prev[:, BL * k : BL * k + BL],
                            rhs=wk_sb[:, o : o + 256],
                            start=(k == 0),
                            stop=False,
                            skip_group_check=True,
                            tile_position=(0, 32 * g),
                        )
                # reset_after: rec_h needs +b_rec_h *before* the r-gate product
                for g in range(NG):
                    nc.tensor.matmul(
                        rec[32 * g : 32 * g + BL, 512:768],
                        lhsT=ones_sb[0:1, 0:BL],
                        rhs=brh_sb[0:1, g * UC : (g + 1) * UC],
                        start=False,
                        stop=True,
                        skip_group_check=True,
                        tile_position=(0, 32 * g),
                    )

                zr = wpool.tile([128, 2 * UC], FP32, name="zr", tag="zr")
                nc.vector.tensor_add(
                    out=zr[:], in0=rec[:, 0 : 2 * UC], in1=xc[:, xo : xo + 2 * UC]
                )
                zrs = wpool.tile([128, 2 * UC], FP32, name="zrs", tag="zrs")
                nc.scalar.activation(out=zrs[:], in_=zr[:], func=AF.Sigmoid)

                t1 = wpool.tile([128, UC], FP32, name="t1", tag="t1")
                nc.vector.tensor_mul(
                    out=t1[:], in0=zrs[:, UC : 2 * UC], in1=rec[:, 2 * UC : 3 * UC]
                )
                t2 = wpool.tile([128, UC], FP32, name="t2", tag="t2")
                nc.vector.tensor_add(
                    out=t2[:], in0=t1[:], in1=xc[:, xo + 2 * UC : xo + 3 * UC]
                )
                hh = wpool.tile([128, UC], FP32, name="hh", tag="hh")
                nc.scalar.activation(out=hh[:], in_=t2[:], func=AF.Tanh)

                d = wpool.tile([128, UC], FP32, name="d", tag="d")
                nc.vector.tensor_sub(out=d[:], in0=h_prev[:], in1=hh[:])
                e = wpool.tile([128, UC], FP32, name="e", tag="e")
                nc.vector.tensor_mul(out=e[:], in0=zrs[:, 0:UC], in1=d[:])
                h_new = spool.tile([128, UC], FP32, name="h_new", tag="h")
                nc.vector.tensor_add(out=h_new[:], in0=hh[:], in1=e[:])
                h16 = wpool.tile([128, UC], FP16, name="h16", tag="h16")
                nc.vector.tensor_copy(out=h16[:], in_=h_new[:])
                # densify to partitions 0..8 so the transpose matmuls need no
                # row-offset tile_position (row-offset stationaries interleaved
                # with the rec matmuls lock up the PE)
                h16d = wpool.tile([BL, U], FP16, name="h16d", tag="h16d")
                for g in range(NG):
                    nc.sync.dma_start(
                        out=h16d[0:BL, g * UC : (g + 1) * UC],
                        in_=h16[32 * g : 32 * g + BL, :],
                    )

                pT = pT_bufs[t % 2]
                for kt in range(KT):
                    nc.tensor.matmul(
                        pT[:, BL * kt : BL * kt + BL],
                        lhsT=h16d[0:BL, 128 * kt : 128 * kt + 128],
                        rhs=idn_sb[0:BL, 0:BL],
                        start=True,
                        stop=True,
                    )
                hT_new = spool.tile([128, KT * BL], FP16, name="hT_new", tag="hT")
                nc.vector.tensor_copy(out=hT_new[:], in_=pT[:])

                for g in range(NG):
                    nc.sync.dma_start(
                        out=out[:, t, g * UC : (g + 1) * UC],
                        in_=h_new[32 * g : 32 * g + BL, :],
                    )
                h_prev, hT_prev = h_new, hT_new
    nc.finalize()
    return nc


def prep_core_inputs(x_l, hidden_l, win_b, wk, brh, n_steps):
    """Per-core host-side layout prep. x_l [BL, n_steps] int32,
    hidden_l [BL, U] f32. win_b/wk/brh shared across cores."""
    nch = n_steps // CH
    # idx[p=dt*8+b, c] = x_l[b, c*16+dt]
    xt_ = np.ascontiguousarray(x_l.T).reshape(nch, CH, BL)          # (c, dt, b)
    idxv = np.ascontiguousarray(xt_.transpose(1, 2, 0).reshape(128, nch)).astype(np.int32)
    # h0s[32g+b, j] = hidden_l[b, 256g+j]
    h0s = np.zeros((128, UC), np.float32)
    h0s.reshape(NG, 32, UC)[:, :BL, :] = hidden_l.reshape(BL, NG, UC).transpose(1, 0, 2)
    # h0t[p, k*8+b] = hidden_l[b, k*128+p]
    h0t = np.ascontiguousarray(
        hidden_l.reshape(BL, KT, 128).transpose(2, 1, 0).reshape(128, KT * BL)
    ).astype(np.float16)
    idn = np.zeros((128, BL), np.float16)
    idn.reshape(NG, 32, BL)[:, :BL, :] = np.eye(BL, dtype=np.float16)
    return {
        "win": win_b,
        "wk": wk,
        "idx": idxv,
        "h0s": h0s,
        "h0t": h0t,
        "idn": idn,
        "brh": brh,
    }


def prep_shared(W_in, W_rec, b_in, b_rec):
    bias = b_in.astype(np.float64) + b_rec.astype(np.float64)
    bias[2 * U :] = b_in[2 * U :]  # h-gate: only b_in folds into the table
    win_b = (W_in.astype(np.float64) + bias[None, :]).astype(np.float32)
    # wk[p, k*3072 + g*768 + gate*256 + j] = W_rec[k*128+p, gate*1024 + g*256 + j]
    wr = W_rec.reshape(KT, 128, 3, NG, UC)
    wk = np.ascontiguousarray(
        wr.transpose(1, 0, 3, 2, 4).reshape(128, KT * G3)
    ).astype(np.float16)
    brh = np.ascontiguousarray(b_rec[2 * U :].reshape(1, U)).astype(np.float16)
    return win_b, wk, brh


_CACHE = {}


def _get_nc(n_steps):
    if n_steps not in _CACHE:
        _CACHE[n_steps] = build_bass(n_steps)
    return _CACHE[n_steps]


def run(x, hidden, W_in, W_rec, b_in, b_rec, n_steps=T, trace=False):
    x = np.asarray(x).astype(np.int32)
    hidden = np.asarray(hidden, dtype=np.float32)
    W_in = np.asarray(W_in, dtype=np.float32)
    W_rec = np.asarray(W_rec, dtype=np.float32)
    b_in = np.asarray(b_in, dtype=np.float32)
    b_rec = np.asarray(b_rec, dtype=np.float32)

    win_b, wk, brh = prep_shared(W_in, W_rec, b_in, b_rec)
    nc = _get_nc(n_steps)
    in_maps = []
    for i in range(NCORES):
        sl = slice(i * BL, (i + 1) * BL)
        in_maps.append(
            prep_core_inputs(x[sl], hidden[sl], win_b, wk, brh, n_steps)
        )
    res = run_bass_kernel_spmd(
        nc, in_maps, core_ids=list(range(NCORES)), trace=trace
    )
    output = np.concatenate([r["out"] for r in res.results], axis=0)
    state = np.ascontiguousarray(output[:, -1, :])
    return (output, state), res


def kernel(x, hidden, W_in, W_rec, b_in, b_rec):
    (output, state), _ = run(x, hidden, W_in, W_rec, b_in, b_rec)
    return output, state


def run_timed(x, hidden, W_in, W_rec, b_in, b_rec, n_steps=T, reps=5):
    """Device-execution wall-clock: inputs pre-staged on device, executable
    cached, min over reps. Mirrors bass2jax.run_bass_via_pjrt's multi-core
    path. Returns ((output, state), per_exec_seconds)."""
    import time

    import jax
    import concourse.mybir as mybir2
    from concourse import bass2jax
    from jax.sharding import Mesh, NamedSharding, PartitionSpec
    from jax.experimental.shard_map import shard_map

    x = np.asarray(x).astype(np.int32)
    hidden = np.asarray(hidden, dtype=np.float32)
    win_b, wk, brh = prep_shared(
        np.asarray(W_in, np.float32), np.asarray(W_rec, np.float32),
        np.asarray(b_in, np.float32), np.asarray(b_rec, np.float32))
    nc = _get_nc(n_steps)
    in_maps = [
        prep_core_inputs(x[i * BL : (i + 1) * BL], hidden[i * BL : (i + 1) * BL],
                         win_b, wk, brh, n_steps)
        for i in range(NCORES)
    ]

    bass2jax.install_neuronx_cc_hook()
    partition_name = nc.partition_id_tensor.name if nc.partition_id_tensor else None
    in_names, out_names, out_avals = [], [], []
    zero_outs = []
    for alloc in nc.m.functions[0].allocations:
        if not isinstance(alloc, mybir2.MemoryLocationSet):
            continue
        name = alloc.memorylocations[0].name
        if alloc.kind == "ExternalInput":
            if name != partition_name:
                in_names.append(name)
        elif alloc.kind == "ExternalOutput":
            out_names.append(name)
            shape = tuple(alloc.tensor_shape)
            dtype = mybir2.dt.np(alloc.dtype)
            out_avals.append(jax.core.ShapedArray(shape, dtype))
            zero_outs.append(np.zeros(shape, dtype))
    n_params = len(in_names)
    all_names = in_names + out_names
    if partition_name is not None:
        all_names = all_names + [partition_name]

    def _body(*args):
        operands = list(args)
        if partition_name is not None:
            operands.append(bass2jax.partition_id_tensor())
        outs = bass2jax._bass_exec_p.bind(
            *operands,
            out_avals=tuple(out_avals),
            in_names=tuple(all_names),
            out_names=tuple(out_names),
            lowering_input_output_aliases=(),
            sim_require_finite=True,
            sim_require_nnan=True,
            nc=nc,
        )
        return tuple(outs)

    devices = jax.devices()[:NCORES]
    mesh = Mesh(np.asarray(devices), ("core",))
    n_outs = len(out_avals)
    sharded = jax.jit(
        shard_map(
            _body, mesh=mesh,
            in_specs=(PartitionSpec("core"),) * (n_params + n_outs),
            out_specs=(PartitionSpec("core"),) * n_outs,
            check_rep=False,
        ),
        donate_argnums=tuple(range(n_params, n_params + n_outs)),
        keep_unused=True,
    )
    sh = NamedSharding(mesh, PartitionSpec("core"))
    dev_in = [
        jax.device_put(
            np.concatenate([np.asarray(in_maps[c][nm]) for c in range(NCORES)], axis=0), sh
        )
        for nm in in_names
    ]
    def fresh_zeros():
        return [
            jax.device_put(np.zeros((NCORES * z.shape[0], *z.shape[1:]), z.dtype), sh)
            for z in zero_outs
        ]

    # compile + warm
    out_arrs = sharded(*dev_in, *fresh_zeros())
    jax.block_until_ready(out_arrs)
    times = []
    for _ in range(reps):
        zs = fresh_zeros()
        jax.block_until_ready(zs)
        t0 = time.perf_counter()
        out_arrs = sharded(*dev_in, *zs)
        jax.block_until_ready(out_arrs)
        times.append(time.perf_counter() - t0)
    oi = out_names.index("out")
    full = np.asarray(out_arrs[oi]).reshape(NCORES, BL, n_steps, U).reshape(B, n_steps, U)
    state = np.ascontiguousarray(full[:, -1, :])
    return (full, state), min(times), times


def np_ref(x, h, W_in, W_rec, b_in, b_rec):
    xp = W_in[x] + b_in
    outs = []
    for t in range(x.shape[1]):
        xt = xp[:, t]
        recv = h @ W_rec + b_rec
        z = 1 / (1 + np.exp(-(xt[:, :U] + recv[:, :U])))
        r = 1 / (1 + np.exp(-(xt[:, U : 2 * U] + recv[:, U : 2 * U])))
        hhat = np.tanh(xt[:, 2 * U :] + r * recv[:, 2 * U :])
        h = z * h + (1 - z) * hhat
        outs.append(h)
    return np.stack(outs, axis=1)


if __name__ == "__main__":
    import sys

    ts = int(sys.argv[1]) if len(sys.argv) > 1 else 16
    rng = np.random.default_rng(0)
    x = rng.integers(0, V, (B, ts)).astype(np.int32)
    hidden = rng.standard_normal((B, U)).astype(np.float32)
    s_in, s_rec = 1.0 / np.sqrt(V), 1.0 / np.sqrt(U)
    W_in = rng.uniform(-s_in, s_in, (V, G3)).astype(np.float32)
    W_rec = rng.uniform(-s_rec, s_rec, (U, G3)).astype(np.float32)
    b_in = rng.uniform(-s_rec, s_rec, (G3,)).astype(np.float32)
    b_rec = rng.uniform(-s_rec, s_rec, (G3,)).astype(np.float32)

    exp = np_ref(x, hidden, W_in, W_rec, b_in, b_rec)
    (got, state), _ = run(x, hidden, W_in, W_rec, b_in, b_rec, n_steps=ts)
    err = np.abs(got - exp)
    rel = np.linalg.norm((got - exp).ravel()) / np.linalg.norm(exp.ravel())
    print("max abs err:", err.max(), "rel:", rel)
    print("PASS" if rel < 5e-3 else "FAIL")
